# revision 13
# baseline (speedup 1.0000x reference)
"""Trainium2 Bass kernel for nn_EnergyModel — fp8(e4m3), range-mask gather.

Only poses with T[:,4:7] inside `ranges` need computing (the rest output the
constant 100000.0, independent of the big tensors) — with randn T that is
~32% of poses.  The host gathers the unmasked poses, folds
c[q,d] = 16*sqrt(2 a_q w_d) into both tensors, quantizes to float8_e4m3, and
remaps features so SBUF partition = f mod 128 and poses pack densely along
the free axis: per core [128, n_poses * 576], chunked ppc poses at a time as
[x-cols | y-cols] halves.

Per chunk (C = ppc*576 cols):
  cols [0, 5120):  TensorE DoubleRow subtract (S=[I|-I]) -> f32 PSUM
  cols [5120, C):  DVE tensor_tensor subtract (fp8e4 -> bf16 SBUF)
  squares (elementwise, no accumulate): ScalarE Square on the PSUM part +
  tail; GPSIMD tensor_tensor mult on 1024 SBUF cols -> one bf16 sq tile
  per-pose energies: DVE segmented tensor_reduce [128, ppc, 576] -> A[:, ...]
  (software-pipelined one chunk behind the squares)
Cross-partition finish: one f32 matmul ones(*2^-8)^T @ A -> [1, n] energies.
"""

import sys

import numpy as np
import ml_dtypes

for _p in ("/opt/trn_rl_repo",):
    if _p not in sys.path:
        sys.path.insert(0, _p)

import concourse.bacc as bacc
import concourse.bass as bass
import concourse.mybir as mybir
from concourse.bass_utils import run_bass_kernel_spmd
from concourse.tile import TileContext

N_CORES = 8
NT, NQ, D = 1024, 128, 576
G = 192
LN2 = 0.6931471805599453
F_TOT = NQ * D
BUMP = 16.0
S_DIM = 576  # feature sub-chunks per partition: f = s*128 + p

_GROUP_DIMS = np.array([1] * 64 + [3] * 64 + [5] * 64)

_cache: dict = {}
_last_in_maps: list | None = None

PUNIT = 1024  # PSUM tile width


def _build(ppc: int, nchunks: int, bufs: int = 3) -> bass.Bass:
    C = ppc * S_DIM
    n_c = ppc * nchunks  # poses per core (padded)
    pe_cols = min(4608, (C * 3 // 4) // 512 * 512)
    dve_cols = C - pe_cols
    gp_cols = dve_cols
    fold_k = min(5, ppc) if ppc > 5 else 0  # poses folded 576->288 by GPSIMD
    units = [1024] * (pe_cols // 1024) + ([512] if pe_cols % 1024 else [])
    f32 = mybir.dt.float32
    bf16 = mybir.dt.bfloat16
    f8 = mybir.dt.float8e4

    nc = bacc.Bacc(
        "TRN2", target_bir_lowering=False, debug=False, num_devices=N_CORES
    )
    zin = nc.declare_dram_parameter(
        "zin", [128, nchunks * 2 * C], f8, isOutput=False
    )
    smat = nc.declare_dram_parameter("smat", [128, 2 * 128], f8, isOutput=False)
    onesv = nc.declare_dram_parameter("onesv", [128, 1], f32, isOutput=False)
    energy = nc.declare_dram_parameter("energy", [1, n_c], f32, isOutput=True)

    with TileContext(nc) as tc:
        with (
            tc.tile_pool(name="io", bufs=bufs) as io,
            tc.tile_pool(name="sq", bufs=2) as sqp,
            tc.tile_pool(name="df", bufs=2) as df,
            tc.tile_pool(name="ps", bufs=2, space="PSUM") as ps,
            tc.tile_pool(name="ps2", bufs=2, space="PSUM") as ps2,
            tc.tile_pool(name="pe", bufs=1, space="PSUM") as pe_pool,
            tc.tile_pool(name="fp", bufs=2) as fp,
            tc.tile_pool(name="acc", bufs=1) as acc,
        ):
            s_t = acc.tile([128, 2 * 128], f8)
            nc.sync.dma_start(out=s_t[:], in_=smat[:])
            sview = s_t[:].rearrange("p (two f) -> p two f", two=2)
            ones_t = acc.tile([128, 1], f32)
            nc.sync.dma_start(out=ones_t[:], in_=onesv[:])
            A = acc.tile([128, n_c], f32)
            e_ps = pe_pool.tile([1, n_c], f32)

            sq_tiles = []
            for c in range(nchunks):
                z_t = io.tile([128, 2 * C], f8, tag="z")
                nc.sync.dma_start(
                    out=z_t[:], in_=zin[:, c * 2 * C : (c + 1) * 2 * C]
                )
                zv = z_t[:].rearrange("p (two f) -> p two f", two=2)
                sq_t = sqp.tile([128, C], bf16, tag="s")

                # DVE subtract for cols [pe_cols, C)
                if dve_cols > 0:
                    diff = df.tile([128, dve_cols], bf16, tag="d")
                    nc.vector.tensor_tensor(
                        diff[:],
                        zv[:, 0, pe_cols:C],
                        zv[:, 1, pe_cols:C],
                        mybir.AluOpType.subtract,
                    )

                # PE subtract -> PSUM, ScalarE squares -> sq tile
                base = 0
                for w in units:
                    pool = ps if w == 1024 else ps2
                    pt = pool.tile([128, w], f32, tag="p")
                    for k in range(w // 512):
                        nc.tensor.matmul(
                            out=pt[:, k * 512 : (k + 1) * 512],
                            lhsT=sview,
                            rhs=zv[:, :, base + k * 512 : base + (k + 1) * 512],
                            start=True,
                            stop=True,
                            perf_mode=mybir.MatmulPerfMode.DoubleRow,
                        )
                    nc.scalar.activation(
                        sq_t[:, base : base + w],
                        pt[:],
                        mybir.ActivationFunctionType.Square,
                        bias=0.0,
                        scale=1.0,
                    )
                    base += w

                if dve_cols > 0:
                    # GPSIMD squares the whole SBUF diff region
                    nc.gpsimd.tensor_tensor(
                        sq_t[:, pe_cols:C],
                        diff[:],
                        diff[:],
                        mybir.AluOpType.mult,
                    )

                sq_tiles.append(sq_t)

                def consume(cc):
                    t = sq_tiles[cc]
                    tv = t[:].rearrange("p (k s) -> p k s", k=ppc)
                    if fold_k:
                        ft = fp.tile([128, fold_k * 288], bf16, tag="f")
                        nc.gpsimd.tensor_tensor(
                            ft[:],
                            tv[:, :fold_k, 0:288],
                            tv[:, :fold_k, 288:576],
                            mybir.AluOpType.add,
                        )
                        nc.vector.tensor_reduce(
                            A[:, cc * ppc : cc * ppc + fold_k],
                            ft[:].rearrange("p (k s) -> p k s", k=fold_k),
                            axis=mybir.AxisListType.X,
                            op=mybir.AluOpType.add,
                        )
                        nc.vector.tensor_reduce(
                            A[:, cc * ppc + fold_k : (cc + 1) * ppc],
                            tv[:, fold_k:, :],
                            axis=mybir.AxisListType.X,
                            op=mybir.AluOpType.add,
                        )
                    else:
                        nc.vector.tensor_reduce(
                            A[:, cc * ppc : (cc + 1) * ppc],
                            tv,
                            axis=mybir.AxisListType.X,
                            op=mybir.AluOpType.add,
                        )
                    # cross-partition finish for this chunk's poses
                    nc.tensor.matmul(
                        out=e_ps[:, cc * ppc : (cc + 1) * ppc],
                        lhsT=ones_t[:],
                        rhs=A[:, cc * ppc : (cc + 1) * ppc],
                        start=True,
                        stop=True,
                    )

                if c > 0:
                    consume(c - 1)

            consume(nchunks - 1)
            e_sb = acc.tile([1, n_c], f32)
            nc.vector.tensor_copy(e_sb[:], e_ps[:])
            nc.sync.dma_start(out=energy[:], in_=e_sb[:])
    nc.finalize()
    return nc


def _softplus64(x: np.ndarray) -> np.ndarray:
    x = np.asarray(x, dtype=np.float64)
    return np.log1p(np.exp(-np.abs(x))) + np.maximum(x, 0.0)


def kernel(T, descriptor, query_feature, query_attention, irrep_weight_logit, ranges):
    descriptor = np.asarray(descriptor)
    query_feature = np.asarray(query_feature)
    a = np.maximum(np.asarray(query_attention, dtype=np.float64), 0.0)
    w_group = _softplus64(irrep_weight_logit) / (LN2 * G)
    w_feat = np.repeat(w_group, _GROUP_DIMS)
    c_qd = (BUMP * np.sqrt(2.0 * a[:, None] * w_feat[None, :])).astype(np.float32)

    # range mask: energy of out-of-range poses is the constant 1e5
    X = np.asarray(T, dtype=np.float32)[:, 4:7]
    rg = np.asarray(ranges, dtype=np.float32)
    in_range = np.all((rg[None, :, 1] >= X) & (X >= rg[None, :, 0]), axis=-1)
    idx = np.nonzero(in_range)[0]
    n = len(idx)

    n_c = max(1, -(-n // N_CORES))  # poses per core
    ppc = min(16, max(1, -(-n_c // 4)))  # poses per chunk
    nchunks = -(-n_c // ppc)
    n_c = ppc * nchunks
    n_pad = n_c * N_CORES

    # gather + quantize only the needed poses
    xs = np.zeros((n_pad, F_TOT), dtype=ml_dtypes.float8_e4m3)
    ys = np.zeros((n_pad, F_TOT), dtype=ml_dtypes.float8_e4m3)
    cf = c_qd.reshape(1, F_TOT)
    xs[:n] = np.clip(
        descriptor.reshape(NT, F_TOT)[idx] * cf, -240.0, 240.0
    ).astype(ml_dtypes.float8_e4m3)
    ys[:n] = np.clip(
        query_feature.reshape(NT, F_TOT)[idx] * cf, -240.0, 240.0
    ).astype(ml_dtypes.float8_e4m3)

    # remap: [n_pad, (s,p)] -> per core [p, chunk, (x|y), k, s]
    C = ppc * S_DIM
    xs = xs.reshape(N_CORES, nchunks, ppc, S_DIM, 128)
    ys = ys.reshape(N_CORES, nchunks, ppc, S_DIM, 128)
    z = np.stack([xs, ys], axis=2)  # [cores, chunks, 2, ppc, s, p]
    z = np.ascontiguousarray(np.moveaxis(z, 5, 2))  # [cores, chunks, p, 2, k, s]
    z = z.reshape(N_CORES, nchunks, 128, 2 * C)
    z = np.ascontiguousarray(np.swapaxes(z, 1, 2)).reshape(
        N_CORES, 128, nchunks * 2 * C
    )

    smat = np.zeros((128, 2, 128), dtype=ml_dtypes.float8_e4m3)
    ii = np.arange(128)
    smat[ii, 0, ii] = 1.0
    smat[ii, 1, ii] = -1.0
    smat = smat.reshape(128, 256)
    onesv = np.full((128, 1), 1.0 / (BUMP * BUMP), dtype=np.float32)

    key = ("mask3", ppc, nchunks)
    nc = _cache.get(key)
    if nc is None:
        nc = _build(ppc, nchunks)
        _cache[key] = nc

    in_maps = [
        {"zin": z[i], "smat": smat, "onesv": onesv} for i in range(N_CORES)
    ]

    global _last_in_maps
    _last_in_maps = in_maps
    res = run_bass_kernel_spmd(nc, in_maps, core_ids=list(range(N_CORES)))
    e_sub = np.concatenate([r["energy"][0] for r in res.results])[:n]

    energy = np.full(NT, 100000.0, dtype=np.float32)
    energy[idx] = e_sub.astype(np.float32)
    return energy


# revision 14
# speedup vs baseline: 1.1023x; 1.1023x over previous
"""Trainium2 Bass kernel for nn_EnergyModel — fp8(e4m3), range-mask gather, v4.

Same op structure as kernel_mask.py (the 44us config) with variable chunk
sizes: a small first chunk so compute starts early and a small last chunk so
the drain tail is short.
"""

import sys

import numpy as np
import ml_dtypes

for _p in ("/opt/trn_rl_repo",):
    if _p not in sys.path:
        sys.path.insert(0, _p)

import concourse.bacc as bacc
import concourse.bass as bass
import concourse.mybir as mybir
from concourse.bass_utils import run_bass_kernel_spmd
from concourse.tile import TileContext

N_CORES = 8
NT, NQ, D = 1024, 128, 576
G = 192
LN2 = 0.6931471805599453
F_TOT = NQ * D
BUMP = 16.0
S_DIM = 576
PUNIT = 1024

_GROUP_DIMS = np.array([1] * 64 + [3] * 64 + [5] * 64)

_cache: dict = {}
_last_in_maps: list | None = None


def _make_chunks(n_c: int) -> list:
    if n_c <= 8:
        return [n_c]
    first, last = 6, 5
    mid = n_c - first - last
    k = max(1, -(-mid // 12))
    base, extra = divmod(mid, k)
    mids = [base + 1] * extra + [base] * (k - extra)
    return [first] + mids + [last]


def _pe_cols(C: int) -> int:
    return min(5120, (C * 13 // 16) // 512 * 512)


def _build(chunks: tuple, bufs: int = 3) -> bass.Bass:
    n_c = sum(chunks)
    C_max = max(chunks) * S_DIM
    dve_max = max(C_max - _pe_cols(C_max), 1)
    f32 = mybir.dt.float32
    bf16 = mybir.dt.bfloat16
    f8 = mybir.dt.float8e4

    nc = bacc.Bacc(
        "TRN2", target_bir_lowering=False, debug=False, num_devices=N_CORES
    )
    zin = nc.declare_dram_parameter(
        "zin", [128, 2 * n_c * S_DIM], f8, isOutput=False
    )
    smat = nc.declare_dram_parameter("smat", [128, 2 * 128], f8, isOutput=False)
    onesv = nc.declare_dram_parameter("onesv", [128, 1], f32, isOutput=False)
    energy = nc.declare_dram_parameter("energy", [1, n_c], f32, isOutput=True)

    with TileContext(nc) as tc:
        with (
            tc.tile_pool(name="io", bufs=bufs) as io,
            tc.tile_pool(name="sq", bufs=2) as sqp,
            tc.tile_pool(name="df", bufs=2) as df,
            tc.tile_pool(name="p1", bufs=2, space="PSUM") as p1,
            tc.tile_pool(name="p2", bufs=2, space="PSUM") as p2,
            tc.tile_pool(name="pe", bufs=1, space="PSUM") as pe_pool,
            tc.tile_pool(name="acc", bufs=1) as acc,
        ):
            # first z chunk early so it overlaps the NEFF preamble
            z0 = io.tile([128, 2 * C_max], f8, tag="z")
            nc.sync.dma_start(out=z0[:, : 2 * chunks[0] * S_DIM], in_=zin[:, : 2 * chunks[0] * S_DIM])

            s_t = acc.tile([128, 2 * 128], f8)
            nc.sync.dma_start(out=s_t[:], in_=smat[:])
            sview = s_t[:].rearrange("p (two f) -> p two f", two=2)
            ones_t = acc.tile([128, 1], f32)
            nc.sync.dma_start(out=ones_t[:], in_=onesv[:])
            A = acc.tile([128, n_c], f32)

            state = []  # (sq_tile, ppc, pose_off)
            zoff = 0
            poff = 0
            for c, ppc in enumerate(chunks):
                C = ppc * S_DIM
                pec = _pe_cols(C)
                dvec = C - pec
                if c == 0:
                    z_t = z0
                else:
                    z_t = io.tile([128, 2 * C_max], f8, tag="z")
                    nc.sync.dma_start(
                        out=z_t[:, : 2 * C], in_=zin[:, zoff : zoff + 2 * C]
                    )
                zv = z_t[:, : 2 * C].rearrange("p (two f) -> p two f", two=2)
                sq_t = sqp.tile([128, C_max], bf16, tag="s")

                if dvec > 0:
                    diff = df.tile([128, dve_max], bf16, tag="d")
                    nc.vector.tensor_tensor(
                        diff[:, :dvec],
                        zv[:, 0, pec:C],
                        zv[:, 1, pec:C],
                        mybir.AluOpType.subtract,
                    )

                base = 0
                units = [1024] * (pec // 1024) + ([512] if pec % 1024 else [])
                for w in units:
                    pool = p1 if w == 1024 else p2
                    pt = pool.tile([128, w], f32, tag="p")
                    for k in range(w // 512):
                        nc.tensor.matmul(
                            out=pt[:, k * 512 : (k + 1) * 512],
                            lhsT=sview,
                            rhs=zv[:, :, base + k * 512 : base + (k + 1) * 512],
                            start=True,
                            stop=True,
                            perf_mode=mybir.MatmulPerfMode.DoubleRow,
                        )
                    nc.scalar.activation(
                        sq_t[:, base : base + w],
                        pt[:],
                        mybir.ActivationFunctionType.Square,
                        bias=0.0,
                        scale=1.0,
                    )
                    base += w

                if dvec > 0:
                    nc.gpsimd.tensor_tensor(
                        sq_t[:, pec:C],
                        diff[:, :dvec],
                        diff[:, :dvec],
                        mybir.AluOpType.mult,
                    )

                state.append((sq_t, ppc, poff))
                poff += ppc
                zoff += 2 * C
                if c > 0:
                    pt_, pk, po = state[c - 1]
                    nc.vector.tensor_reduce(
                        A[:, po : po + pk],
                        pt_[:, : pk * S_DIM].rearrange("p (k s) -> p k s", k=pk),
                        axis=mybir.AxisListType.X,
                        op=mybir.AluOpType.add,
                    )

            pt_, pk, po = state[-1]
            nc.vector.tensor_reduce(
                A[:, po : po + pk],
                pt_[:, : pk * S_DIM].rearrange("p (k s) -> p k s", k=pk),
                axis=mybir.AxisListType.X,
                op=mybir.AluOpType.add,
            )

            e_ps = pe_pool.tile([1, n_c], f32)
            nc.tensor.matmul(
                out=e_ps[:], lhsT=ones_t[:], rhs=A[:], start=True, stop=True
            )
            e_sb = acc.tile([1, n_c], f32)
            nc.vector.tensor_copy(e_sb[:], e_ps[:])
            nc.sync.dma_start(out=energy[:], in_=e_sb[:])
    nc.finalize()
    return nc


def _softplus64(x: np.ndarray) -> np.ndarray:
    x = np.asarray(x, dtype=np.float64)
    return np.log1p(np.exp(-np.abs(x))) + np.maximum(x, 0.0)


def kernel(T, descriptor, query_feature, query_attention, irrep_weight_logit, ranges):
    descriptor = np.asarray(descriptor)
    query_feature = np.asarray(query_feature)
    a = np.maximum(np.asarray(query_attention, dtype=np.float64), 0.0)
    w_group = _softplus64(irrep_weight_logit) / (LN2 * G)
    w_feat = np.repeat(w_group, _GROUP_DIMS)
    c_qd = (BUMP * np.sqrt(2.0 * a[:, None] * w_feat[None, :])).astype(np.float32)

    X = np.asarray(T, dtype=np.float32)[:, 4:7]
    rg = np.asarray(ranges, dtype=np.float32)
    in_range = np.all((rg[None, :, 1] >= X) & (X >= rg[None, :, 0]), axis=-1)
    idx = np.nonzero(in_range)[0]
    n = len(idx)

    n_c = max(1, -(-n // N_CORES))
    chunks = tuple(_make_chunks(n_c))
    n_pad = n_c * N_CORES

    xs = np.zeros((n_pad, F_TOT), dtype=ml_dtypes.float8_e4m3)
    ys = np.zeros((n_pad, F_TOT), dtype=ml_dtypes.float8_e4m3)
    cf = c_qd.reshape(1, F_TOT)
    xs[:n] = np.clip(
        descriptor.reshape(NT, F_TOT)[idx] * cf, -240.0, 240.0
    ).astype(ml_dtypes.float8_e4m3)
    ys[:n] = np.clip(
        query_feature.reshape(NT, F_TOT)[idx] * cf, -240.0, 240.0
    ).astype(ml_dtypes.float8_e4m3)

    # remap per core: partition = f % 128, pose-cols grouped per chunk as
    # [x-half | y-half]
    xs = xs.reshape(N_CORES, n_c, S_DIM, 128)
    ys = ys.reshape(N_CORES, n_c, S_DIM, 128)
    zparts = []
    poff = 0
    for ppc in chunks:
        xc = xs[:, poff : poff + ppc]  # [cores, ppc, s, p]
        yc = ys[:, poff : poff + ppc]
        blk = np.stack([xc, yc], axis=1)  # [cores, 2, ppc, s, p]
        blk = np.moveaxis(blk, 4, 1)  # [cores, p, 2, ppc, s]
        zparts.append(blk.reshape(N_CORES, 128, 2 * ppc * S_DIM))
        poff += ppc
    z = np.ascontiguousarray(np.concatenate(zparts, axis=2))

    smat = np.zeros((128, 2, 128), dtype=ml_dtypes.float8_e4m3)
    ii = np.arange(128)
    smat[ii, 0, ii] = 1.0
    smat[ii, 1, ii] = -1.0
    smat = smat.reshape(128, 256)
    onesv = np.full((128, 1), 1.0 / (BUMP * BUMP), dtype=np.float32)

    key = ("mask4", chunks)
    nc = _cache.get(key)
    if nc is None:
        nc = _build(chunks)
        _cache[key] = nc

    in_maps = [
        {"zin": z[i], "smat": smat, "onesv": onesv} for i in range(N_CORES)
    ]

    global _last_in_maps
    _last_in_maps = in_maps
    res = run_bass_kernel_spmd(nc, in_maps, core_ids=list(range(N_CORES)))
    e_sub = np.concatenate([r["energy"][0] for r in res.results])[:n]

    energy = np.full(NT, 100000.0, dtype=np.float32)
    energy[idx] = e_sub.astype(np.float32)
    return energy


# revision 15
# speedup vs baseline: 1.1243x; 1.0200x over previous
"""Trainium2 Bass kernel for nn_EnergyModel — fp8(e4m3), range-mask gather.

Only poses with T[:,4:7] inside `ranges` need computing (the rest output the
constant 100000.0, independent of the big tensors) — with randn T that is
~32% of poses.  The host gathers the unmasked poses, folds
c[q,d] = 16*sqrt(2 a_q w_d) into both tensors, quantizes to float8_e4m3, and
remaps features so SBUF partition = f mod 128 and poses pack densely along
the free axis: per core [128, n_poses * 576], chunked ppc poses at a time as
[x-cols | y-cols] halves.

Per chunk (C = ppc*576 cols):
  cols [0, 5120):  TensorE DoubleRow subtract (S=[I|-I]) -> f32 PSUM
  cols [5120, C):  DVE tensor_tensor subtract (fp8e4 -> bf16 SBUF)
  squares (elementwise, no accumulate): ScalarE Square on the PSUM part +
  tail; GPSIMD tensor_tensor mult on 1024 SBUF cols -> one bf16 sq tile
  per-pose energies: DVE segmented tensor_reduce [128, ppc, 576] -> A[:, ...]
  (software-pipelined one chunk behind the squares)
Cross-partition finish: one f32 matmul ones(*2^-8)^T @ A -> [1, n] energies.
"""

import sys

import numpy as np
import ml_dtypes

for _p in ("/opt/trn_rl_repo",):
    if _p not in sys.path:
        sys.path.insert(0, _p)

import concourse.bacc as bacc
import concourse.bass as bass
import concourse.mybir as mybir
from concourse.bass_utils import run_bass_kernel_spmd
from concourse.tile import TileContext

N_CORES = 8
NT, NQ, D = 1024, 128, 576
G = 192
LN2 = 0.6931471805599453
F_TOT = NQ * D
BUMP = 16.0
S_DIM = 576  # feature sub-chunks per partition: f = s*128 + p

_GROUP_DIMS = np.array([1] * 64 + [3] * 64 + [5] * 64)

_cache: dict = {}
_last_in_maps: list | None = None

PUNIT = 1024  # PSUM tile width


def _build(ppc: int, nchunks: int, bufs: int = 3) -> bass.Bass:
    C = ppc * S_DIM
    n_c = ppc * nchunks  # poses per core (padded)
    pe_cols = min(5120, (C // PUNIT) * PUNIT)
    npunits = pe_cols // PUNIT
    dve_cols = C - pe_cols
    gp_cols = min(1024, dve_cols)
    f32 = mybir.dt.float32
    bf16 = mybir.dt.bfloat16
    f8 = mybir.dt.float8e4

    nc = bacc.Bacc(
        "TRN2", target_bir_lowering=False, debug=False, num_devices=N_CORES
    )
    zin = nc.declare_dram_parameter(
        "zin", [128, nchunks * 2 * C], f8, isOutput=False
    )
    smat = nc.declare_dram_parameter("smat", [128, 2 * 128], f8, isOutput=False)
    onesv = nc.declare_dram_parameter("onesv", [128, 1], f32, isOutput=False)
    energy = nc.declare_dram_parameter("energy", [1, n_c], f32, isOutput=True)

    with TileContext(nc) as tc:
        with (
            tc.tile_pool(name="io", bufs=bufs) as io,
            tc.tile_pool(name="sq", bufs=2) as sqp,
            tc.tile_pool(name="df", bufs=2) as df,
            tc.tile_pool(name="ps", bufs=3, space="PSUM") as ps,
            tc.tile_pool(name="pe", bufs=1, space="PSUM") as pe_pool,
            tc.tile_pool(name="acc", bufs=1) as acc,
        ):
            z0 = io.tile([128, 2 * C], f8, tag="z")
            nc.sync.dma_start(out=z0[:], in_=zin[:, : 2 * C])
            s_t = acc.tile([128, 2 * 128], f8)
            nc.sync.dma_start(out=s_t[:], in_=smat[:])
            sview = s_t[:].rearrange("p (two f) -> p two f", two=2)
            ones_t = acc.tile([128, 1], f32)
            nc.sync.dma_start(out=ones_t[:], in_=onesv[:])
            A = acc.tile([128, n_c], f32)

            sq_tiles = []
            for c in range(nchunks):
                if c == 0:
                    z_t = z0
                else:
                    z_t = io.tile([128, 2 * C], f8, tag="z")
                    nc.sync.dma_start(
                        out=z_t[:], in_=zin[:, c * 2 * C : (c + 1) * 2 * C]
                    )
                zv = z_t[:].rearrange("p (two f) -> p two f", two=2)
                sq_t = sqp.tile([128, C], bf16, tag="s")

                # DVE subtract for cols [pe_cols, C)
                if dve_cols > 0:
                    diff = df.tile([128, dve_cols], bf16, tag="d")
                    nc.vector.tensor_tensor(
                        diff[:],
                        zv[:, 0, pe_cols:C],
                        zv[:, 1, pe_cols:C],
                        mybir.AluOpType.subtract,
                    )

                # PE subtract -> PSUM, ScalarE squares -> sq tile
                for u in range(npunits):
                    base = u * PUNIT
                    pt = ps.tile([128, PUNIT], f32, tag="p")
                    for k in range(PUNIT // 512):
                        nc.tensor.matmul(
                            out=pt[:, k * 512 : (k + 1) * 512],
                            lhsT=sview,
                            rhs=zv[:, :, base + k * 512 : base + (k + 1) * 512],
                            start=True,
                            stop=True,
                            perf_mode=mybir.MatmulPerfMode.DoubleRow,
                        )
                    nc.scalar.activation(
                        sq_t[:, base : base + PUNIT],
                        pt[:],
                        mybir.ActivationFunctionType.Square,
                        bias=0.0,
                        scale=1.0,
                    )

                if dve_cols > 0:
                    # GPSIMD squares gp_cols of the SBUF diff
                    nc.gpsimd.tensor_tensor(
                        sq_t[:, pe_cols : pe_cols + gp_cols],
                        diff[:, :gp_cols],
                        diff[:, :gp_cols],
                        mybir.AluOpType.mult,
                    )
                    if gp_cols < dve_cols:
                        nc.scalar.activation(
                            sq_t[:, pe_cols + gp_cols : C],
                            diff[:, gp_cols:],
                            mybir.ActivationFunctionType.Square,
                            bias=0.0,
                            scale=1.0,
                        )

                sq_tiles.append(sq_t)
                # software-pipelined segmented reduce (one chunk behind)
                if c > 0:
                    prev = sq_tiles[c - 1]
                    nc.vector.tensor_reduce(
                        A[:, (c - 1) * ppc : c * ppc],
                        prev[:].rearrange("p (k s) -> p k s", k=ppc),
                        axis=mybir.AxisListType.X,
                        op=mybir.AluOpType.add,
                    )

            nc.vector.tensor_reduce(
                A[:, (nchunks - 1) * ppc : nchunks * ppc],
                sq_tiles[-1][:].rearrange("p (k s) -> p k s", k=ppc),
                axis=mybir.AxisListType.X,
                op=mybir.AluOpType.add,
            )

            # cross-partition: energy[1, n_c] = (ones*inv2)^T @ A
            e_ps = pe_pool.tile([1, n_c], f32)
            nc.tensor.matmul(
                out=e_ps[:], lhsT=ones_t[:], rhs=A[:], start=True, stop=True
            )
            e_sb = acc.tile([1, n_c], f32)
            nc.scalar.copy(e_sb[:], e_ps[:])
            nc.sync.dma_start(out=energy[:], in_=e_sb[:])
    nc.finalize()
    return nc


def _softplus64(x: np.ndarray) -> np.ndarray:
    x = np.asarray(x, dtype=np.float64)
    return np.log1p(np.exp(-np.abs(x))) + np.maximum(x, 0.0)


def kernel(T, descriptor, query_feature, query_attention, irrep_weight_logit, ranges):
    descriptor = np.asarray(descriptor)
    query_feature = np.asarray(query_feature)
    a = np.maximum(np.asarray(query_attention, dtype=np.float64), 0.0)
    w_group = _softplus64(irrep_weight_logit) / (LN2 * G)
    w_feat = np.repeat(w_group, _GROUP_DIMS)
    c_qd = (BUMP * np.sqrt(2.0 * a[:, None] * w_feat[None, :])).astype(np.float32)

    # range mask: energy of out-of-range poses is the constant 1e5
    X = np.asarray(T, dtype=np.float32)[:, 4:7]
    rg = np.asarray(ranges, dtype=np.float32)
    in_range = np.all((rg[None, :, 1] >= X) & (X >= rg[None, :, 0]), axis=-1)
    idx = np.nonzero(in_range)[0]
    n = len(idx)

    n_c = max(1, -(-n // N_CORES))  # poses per core
    ppc = min(16, max(1, -(-n_c // 4)))  # poses per chunk
    nchunks = -(-n_c // ppc)
    n_c = ppc * nchunks
    n_pad = n_c * N_CORES

    # gather + quantize only the needed poses
    xs = np.zeros((n_pad, F_TOT), dtype=ml_dtypes.float8_e4m3)
    ys = np.zeros((n_pad, F_TOT), dtype=ml_dtypes.float8_e4m3)
    cf = c_qd.reshape(1, F_TOT)
    xs[:n] = np.clip(
        descriptor.reshape(NT, F_TOT)[idx] * cf, -240.0, 240.0
    ).astype(ml_dtypes.float8_e4m3)
    ys[:n] = np.clip(
        query_feature.reshape(NT, F_TOT)[idx] * cf, -240.0, 240.0
    ).astype(ml_dtypes.float8_e4m3)

    # remap: [n_pad, (s,p)] -> per core [p, chunk, (x|y), k, s]
    C = ppc * S_DIM
    xs = xs.reshape(N_CORES, nchunks, ppc, S_DIM, 128)
    ys = ys.reshape(N_CORES, nchunks, ppc, S_DIM, 128)
    z = np.stack([xs, ys], axis=2)  # [cores, chunks, 2, ppc, s, p]
    z = np.ascontiguousarray(np.moveaxis(z, 5, 2))  # [cores, chunks, p, 2, k, s]
    z = z.reshape(N_CORES, nchunks, 128, 2 * C)
    z = np.ascontiguousarray(np.swapaxes(z, 1, 2)).reshape(
        N_CORES, 128, nchunks * 2 * C
    )

    smat = np.zeros((128, 2, 128), dtype=ml_dtypes.float8_e4m3)
    ii = np.arange(128)
    smat[ii, 0, ii] = 1.0
    smat[ii, 1, ii] = -1.0
    smat = smat.reshape(128, 256)
    onesv = np.full((128, 1), 1.0 / (BUMP * BUMP), dtype=np.float32)

    key = ("mask5", ppc, nchunks)
    nc = _cache.get(key)
    if nc is None:
        nc = _build(ppc, nchunks)
        _cache[key] = nc

    in_maps = [
        {"zin": z[i], "smat": smat, "onesv": onesv} for i in range(N_CORES)
    ]

    global _last_in_maps
    _last_in_maps = in_maps
    res = run_bass_kernel_spmd(nc, in_maps, core_ids=list(range(N_CORES)))
    e_sub = np.concatenate([r["energy"][0] for r in res.results])[:n]

    energy = np.full(NT, 100000.0, dtype=np.float32)
    energy[idx] = e_sub.astype(np.float32)
    return energy
